# revision 1
# baseline (speedup 1.0000x reference)
"""Llama RoPE attention (B=2, S=2048, H=2048, 16 heads) on 8 NeuronCores.

Tensor-parallel over heads: core m owns heads {2m, 2m+1}. Each core gets the
full activation x (transposed host-side to [HIDDEN, B*S] so the contraction
dim lands on SBUF partitions with contiguous DMA) plus its 256-column slice
of wq/wk/wv (also host-transposed). On-chip per core:

  phase 1: q_T/k_T = (w.T)^T @ x_T accumulated over 16 k-tiles (fp32r
           matmuls), RoPE applied out of PSUM on the vector engine;
           v kept in natural [n, d] layout (x-tile stationary) for PV.
  phase 2: per (batch, head): flash-style over sq blocks of 512:
           scores_T[sk, sq] = k_T.T @ q_T (one matmul per sk tile),
           exp on scalar engine (fused 1/sqrt(d) scale) -> fp16 e tiles,
           PV accumulates v.T @ e over sk in PSUM, softmax denominator
           accumulates ones.T @ e in PSUM, then out = PV * (1/denom)
           broadcast via a K=1 matmul.

Output is the transposed flattened attention output [256, 4096] per core;
the host stacks core outputs and transposes back.
"""

import math
import os
import sys

for _p in ("/opt/trn_rl_repo", "/root/.axon_site/_ro/trn_rl_repo"):
    if os.path.isdir(_p) and _p not in sys.path:
        sys.path.insert(0, _p)
        break

import numpy as np

import concourse.bass as bass
import concourse.bacc as bacc
import concourse.mybir as mybir
from concourse import bass_isa, tile
from concourse.bass_utils import run_bass_kernel_spmd

N_CORES = 8
HIDDEN = 2048
N_HEAD = 16
HEAD_DIM = 128
B = 2
S = 2048
NTOK = B * S  # 4096
OPC = 256  # output cols per core = 2 heads * 128
KI = HIDDEN // 128  # 16 contraction tiles
NB = NTOK // 512  # 8 n-blocks of 512 tokens
NBLK = 512
SCALE = 1.0 / math.sqrt(HEAD_DIM)
F32 = mybir.dt.float32
F16 = mybir.dt.float16
F32R = mybir.dt.float32r
EXP = mybir.ActivationFunctionType.Exp

_CACHE = {}

# test.py can read this after calling kernel() with BASS_TRACE=1
LAST_RESULT = None


def _build_nc():
    nc = bacc.Bacc("TRN2", target_bir_lowering=False, debug=False,
                   num_devices=N_CORES)
    xT = nc.dram_tensor("xT", [HIDDEN, NTOK], F32R, kind="ExternalInput")
    wqT = nc.dram_tensor("wqT", [HIDDEN, OPC], F32R, kind="ExternalInput")
    wkT = nc.dram_tensor("wkT", [HIDDEN, OPC], F32R, kind="ExternalInput")
    wvT = nc.dram_tensor("wvT", [HIDDEN, OPC], F32R, kind="ExternalInput")
    cosT = nc.dram_tensor("cosT", [HEAD_DIM, S], F32, kind="ExternalInput")
    sinS = nc.dram_tensor("sinS", [HEAD_DIM, S], F32, kind="ExternalInput")
    outT = nc.dram_tensor("outT", [OPC, NTOK], F32, kind="ExternalOutput")

    with tile.TileContext(nc) as tc:
        with (
            tc.tile_pool(name="const", bufs=1) as cp,
            tc.tile_pool(name="qk_res", bufs=1) as qkp,
            tc.tile_pool(name="v_res", bufs=1) as vp,
        ):
            # resident phase-1 outputs
            q_sb = [qkp.tile([128, NTOK], F32R, tag=f"q{o}", name=f"q_sb{o}") for o in range(2)]
            k_sb = [qkp.tile([128, NTOK], F32R, tag=f"k{o}", name=f"k_sb{o}") for o in range(2)]
            v_sb = vp.tile([128, 32 * 256], F16, tag="v")  # [n%128, (nblk d)]

            cos_sb = cp.tile([128, S], F32, tag="cos")
            sin_sb = cp.tile([128, S], F32, tag="sin")

            # ---------------- phase 1: projections + RoPE ----------------
            with (
                tc.tile_pool(name="w", bufs=1) as wp,
                tc.tile_pool(name="x", bufs=6) as xp,
                tc.tile_pool(name="rope_tmp", bufs=3) as rtp,
                tc.tile_pool(name="ps_qk", bufs=4, space="PSUM") as psqk,
                tc.tile_pool(name="ps_v", bufs=2, space="PSUM") as psv,
            ):
                def load_w(nm, drt):
                    t = wp.tile([128, KI * 256], F32R, tag=f"w{nm}",
                                name=f"w_{nm}")
                    nc.sync.dma_start(
                        t[:].rearrange("p (t o) -> p t o", t=KI),
                        drt[:, :].rearrange("(t p) o -> p t o", p=128),
                    )
                    return t

                def load_x(nb):
                    n0 = nb * NBLK
                    xc = []
                    for c in range(4):
                        xt = xp.tile([128, 4 * NBLK], F32R, tag="x",
                                     name="xt")
                        nc.sync.dma_start(
                            xt[:].rearrange("p (t n) -> p t n", t=4),
                            xT[c * 512:(c + 1) * 512, n0:n0 + NBLK]
                            .rearrange("(t p) n -> p t n", p=128),
                        )
                        xc.append(xt)
                    return xc

                # DMA order: wq + first x block first so the PE starts
                # ~10us in instead of ~35us; cos/sin only gate RoPE.
                w_sb = {"q": load_w("q", wqT)}
                xc0 = load_x(0)
                w_sb["k"] = load_w("k", wkT)
                w_sb["v"] = load_w("v", wvT)
                nc.sync.dma_start(cos_sb[:], cosT[:, :])
                nc.sync.dma_start(sin_sb[:], sinS[:, :])

                for nb in range(NB):
                    n0 = nb * NBLK
                    s0 = (nb % 4) * NBLK  # in-batch position offset
                    xc = xc0 if nb == 0 else load_x(nb)

                    for nm, outsb in (("q", q_sb), ("k", k_sb)):
                        for o in range(2):
                            pq = psqk.tile([128, NBLK], F32, tag="pqk")
                            for c in range(4):
                                for t in range(4):
                                    i = c * 4 + t
                                    nc.tensor.matmul(
                                        pq[:],
                                        w_sb[nm][:, i * 256 + o * 128:
                                                 i * 256 + o * 128 + 128]
                                        ,
                                        xc[c][:, t * NBLK:(t + 1) * NBLK]
                                        ,
                                        start=(i == 0), stop=(i == KI - 1),
                                    )
                            # RoPE: out = pq*cos + rot(pq)*sinS
                            t1 = rtp.tile([128, NBLK], F32, tag="t1")
                            nc.vector.tensor_mul(
                                t1[:], pq[:], cos_sb[:, s0:s0 + NBLK])
                            dst = outsb[o][:, n0:n0 + NBLK]
                            nc.vector.tensor_mul(
                                dst[0:64, :], pq[64:128, :],
                                sin_sb[0:64, s0:s0 + NBLK])
                            nc.vector.tensor_mul(
                                dst[64:128, :], pq[0:64, :],
                                sin_sb[64:128, s0:s0 + NBLK])
                            nc.vector.tensor_add(dst[:, :], dst[:, :], t1[:])

                    # v in natural layout: lhsT = x tile, rhs = wv
                    for j in range(4):
                        pv = psv.tile([128, 256], F32, tag="pv")
                        for c in range(4):
                            for t in range(4):
                                i = c * 4 + t
                                nc.tensor.matmul(
                                    pv[:],
                                    xc[c][:, t * NBLK + j * 128:
                                          t * NBLK + j * 128 + 128]
                                    ,
                                    w_sb["v"][:, i * 256:(i + 1) * 256]
                                    ,
                                    start=(i == 0), stop=(i == KI - 1),
                                )
                        jg = nb * 4 + j
                        nc.scalar.copy(v_sb[:, jg * 256:(jg + 1) * 256], pv[:])

            # ---------------- phase 2: attention ----------------
            # sk outer / sq inner: one k/v LDWEIGHTS serves 4 matmuls, the
            # softmax denominator accumulates on DVE (fp16) instead of PE,
            # partition-reduced + broadcast on GpSimd.  PSUM: 2x scores
            # tiles [128,1024] (4 banks) + 4 output accumulators (4 banks).
            with (
                tc.tile_pool(name="ps_s", bufs=2, space="PSUM") as pss,
                tc.tile_pool(name="ps_o", bufs=1, space="PSUM") as pso,
                tc.tile_pool(name="e", bufs=4) as ep,
                tc.tile_pool(name="acc", bufs=2) as accp,
                tc.tile_pool(name="att_tmp", bufs=2) as atp,
            ):
                for b in range(2):
                    for h in range(2):
                        sq0 = b * 2048
                        po = [pso.tile([128, NBLK], F32, tag=f"po{q}",
                                       name=f"po{q}") for q in range(4)]
                        acc = [accp.tile([128, 2 * NBLK], F16, tag=f"acc{a}",
                                         name=f"acc{a}") for a in range(2)]
                        for sk in range(16):
                            kt = k_sb[h][:, b * 2048 + sk * 128:
                                         b * 2048 + sk * 128 + 128]
                            jg = b * 16 + sk
                            vt = v_sb[:, jg * 256 + h * 128:
                                      jg * 256 + h * 128 + 128]
                            es = []
                            for half in range(2):
                                ps = pss.tile([128, 2 * NBLK], F32, tag="ps")
                                for q in range(2):
                                    nc.tensor.matmul(
                                        ps[:, q * NBLK:(q + 1) * NBLK],
                                        kt,
                                        q_sb[h][:, sq0 + (half * 2 + q) * NBLK:
                                                sq0 + (half * 2 + q + 1) * NBLK],
                                        start=True, stop=True,
                                    )
                                e = ep.tile([128, 2 * NBLK], F16, tag="e")
                                nc.scalar.activation(e[:], ps[:], EXP,
                                                     scale=SCALE)
                                es.append(e)
                                # denominator partial sums on DVE (fp16 2x)
                                if sk == 0:
                                    nc.vector.tensor_copy(acc[half][:], e[:])
                                else:
                                    nc.vector.tensor_add(
                                        acc[half][:], acc[half][:], e[:])
                            for q in range(4):
                                nc.tensor.matmul(
                                    po[q][:],
                                    vt,
                                    es[q // 2][:, (q % 2) * NBLK:
                                               (q % 2 + 1) * NBLK],
                                    start=(sk == 0), stop=(sk == 15),
                                )
                        # denom = sum over partitions; recip spread over
                        # 128 lanes via an SBUF repartition DMA round-trip
                        for half in range(2):
                            ar = atp.tile([128, 2 * NBLK], F32,
                                          tag="ar", name="ar")
                            nc.gpsimd.partition_all_reduce(
                                ar[:], acc[half][:], channels=128,
                                reduce_op=bass_isa.ReduceOp.add)
                            rp = atp.tile([128, 8], F32, tag="rp", name="rp")
                            nc.sync.dma_start(rp[:], ar[0:1, :])
                            rc = atp.tile([128, 8], F32, tag="rc", name="rc")
                            nc.vector.reciprocal(rc[:], rp[:])
                            rrow = atp.tile([1, 2 * NBLK], F32,
                                            tag="rrow", name="rrow")
                            nc.sync.dma_start(rrow[:], rc[:])
                            bc = atp.tile([128, 2 * NBLK], F32,
                                          tag="bc", name="bc")
                            nc.gpsimd.partition_broadcast(bc[:], rrow[:])
                            for q in range(2):
                                qq = half * 2 + q
                                osb = atp.tile([128, NBLK], F32,
                                               tag="osb", name="osb")
                                nc.vector.tensor_mul(
                                    osb[:], po[qq][:],
                                    bc[:, q * NBLK:(q + 1) * NBLK])
                                nc.sync.dma_start(
                                    outT[h * 128:(h + 1) * 128,
                                         sq0 + qq * NBLK:sq0 + (qq + 1) * NBLK],
                                    osb[:])
    nc.compile()
    return nc


def _get_nc():
    if "nc" not in _CACHE:
        _CACHE["nc"] = _build_nc()
    return _CACHE["nc"]


def _cos_sin():
    if "cs" not in _CACHE:
        half = np.arange(0, HEAD_DIM, 2, dtype=np.float32)[: HEAD_DIM // 2]
        freq = (1.0 / 10000.0 ** (half / HEAD_DIM)).astype(np.float32)
        t = np.arange(S, dtype=np.float32)
        freqs = np.outer(t, freq).astype(np.float32)  # [S, 64]
        emb = np.concatenate([freqs, freqs], axis=1)  # [S, 128]
        cosT = np.ascontiguousarray(np.cos(emb).astype(np.float32).T)
        sinT = np.ascontiguousarray(np.sin(emb).astype(np.float32).T)
        sinS = np.concatenate([-sinT[0:64], sinT[64:128]], axis=0)
        _CACHE["cs"] = (cosT, np.ascontiguousarray(sinS))
    return _CACHE["cs"]


def kernel(x, wq, wk, wv):
    global LAST_RESULT
    nc = _get_nc()
    cosT, sinS = _cos_sin()
    x2 = np.ascontiguousarray(
        x.reshape(NTOK, HIDDEN).T).astype(np.float32)  # [HIDDEN, NTOK]
    in_maps = []
    for m in range(N_CORES):
        sl = slice(m * OPC, (m + 1) * OPC)
        in_maps.append({
            "xT": x2,
            "wqT": np.ascontiguousarray(np.asarray(wq)[sl].T),
            "wkT": np.ascontiguousarray(np.asarray(wk)[sl].T),
            "wvT": np.ascontiguousarray(np.asarray(wv)[sl].T),
            "cosT": cosT,
            "sinS": sinS,
        })
    res = run_bass_kernel_spmd(nc, in_maps, core_ids=list(range(N_CORES)))
    LAST_RESULT = res
    big = np.concatenate([r["outT"] for r in res.results], axis=0)
    return np.ascontiguousarray(big.T).reshape(B, S, HIDDEN).astype(np.float32)


if __name__ == "__main__":
    _get_nc()
    print("build OK")



# revision 4
# speedup vs baseline: 1.1244x; 1.1244x over previous
"""Llama RoPE attention (B=2, S=2048, H=2048, 16 heads) on 8 NeuronCores.

Tensor-parallel over heads: core m owns heads {2m, 2m+1}. Each core gets the
full activation x (transposed host-side to [HIDDEN, B*S], cast fp16) plus its
256-column slice of wq/wk/wv (host-transposed, fp16). On-chip per core:

  phase 1: q_T/k_T = (w.T)^T @ x_T accumulated over 16 k-tiles (fp16
           matmuls), RoPE applied out of PSUM on the vector engine (fp16
           out); v kept in natural [n, d] fp16 layout for PV.
  phase 2: per (batch, head): sk outer / sq inner over 512-blocks:
           scores_T[sk, sq] = k_T.T @ q_T, exp on scalar engine (fused
           1/sqrt(d) scale) -> fp16 e tiles, PV accumulates v.T @ e over sk
           in PSUM, softmax denominator partial sums accumulate on DVE
           (fp16); the partition reduction of the denominator is 4 tiny
           ones.T @ acc matmuls on the PE (partition offsets 0/32/64/96 of
           one PSUM tile), then reciprocal on DVE, partition_broadcast on
           GpSimd, final scale + store. PE never waits on GpSimd.

Output is the transposed flattened attention output [256, 4096] per core;
the host stacks core outputs and transposes back.
"""

import math
import os
import sys

for _p in ("/opt/trn_rl_repo", "/root/.axon_site/_ro/trn_rl_repo"):
    if os.path.isdir(_p) and _p not in sys.path:
        sys.path.insert(0, _p)
        break

import numpy as np

import concourse.bass as bass
import concourse.bacc as bacc
import concourse.mybir as mybir
from concourse import bass_isa, tile
from concourse.bass_utils import run_bass_kernel_spmd

N_CORES = 8
HIDDEN = 2048
N_HEAD = 16
HEAD_DIM = 128
B = 2
S = 2048
NTOK = B * S  # 4096
OPC = 256  # output cols per core = 2 heads * 128
KI = HIDDEN // 128  # 16 contraction tiles
NB = NTOK // 512  # 8 n-blocks of 512 tokens
NBLK = 512
SCALE = 1.0 / math.sqrt(HEAD_DIM)
F32 = mybir.dt.float32
F16 = mybir.dt.float16
EXP = mybir.ActivationFunctionType.Exp

_CACHE = {}

# test.py can read this after calling kernel() with BASS_TRACE=1
LAST_RESULT = None


def _build_nc():
    nc = bacc.Bacc("TRN2", target_bir_lowering=False, debug=False,
                   num_devices=N_CORES)
    xT = nc.dram_tensor("xT", [HIDDEN, NTOK], F16, kind="ExternalInput")
    wqT = nc.dram_tensor("wqT", [HIDDEN, OPC], F16, kind="ExternalInput")
    wkT = nc.dram_tensor("wkT", [HIDDEN, OPC], F16, kind="ExternalInput")
    wvT = nc.dram_tensor("wvT", [HIDDEN, OPC], F16, kind="ExternalInput")
    cosT = nc.dram_tensor("cosT", [HEAD_DIM, S], F32, kind="ExternalInput")
    sinS = nc.dram_tensor("sinS", [HEAD_DIM, S], F32, kind="ExternalInput")
    ones_in = nc.dram_tensor("ones_in", [128, 8], F16, kind="ExternalInput")
    outT = nc.dram_tensor("outT", [OPC, NTOK], F32, kind="ExternalOutput")

    with tile.TileContext(nc) as tc:
        with (
            tc.tile_pool(name="const", bufs=1) as cp,
            tc.tile_pool(name="qk_res", bufs=1) as qkp,
            tc.tile_pool(name="v_res", bufs=1) as vp,
        ):
            # resident phase-1 outputs (fp16)
            q_sb = [qkp.tile([128, NTOK], F16, tag=f"q{o}", name=f"q_sb{o}")
                    for o in range(2)]
            k_sb = [qkp.tile([128, NTOK], F16, tag=f"k{o}", name=f"k_sb{o}")
                    for o in range(2)]
            v_sb = vp.tile([128, 32 * 256], F16, tag="v")  # [n%128, (nblk d)]

            cos_sb = cp.tile([128, S], F32, tag="cos")
            sin_sb = cp.tile([128, S], F32, tag="sin")
            ones_sb = cp.tile([128, 8], F16, tag="ones")

            # ---------------- phase 1: projections + RoPE ----------------
            with (
                tc.tile_pool(name="w", bufs=1) as wp,
                tc.tile_pool(name="x", bufs=6) as xp,
                tc.tile_pool(name="rope_tmp", bufs=3) as rtp,
                tc.tile_pool(name="ps_qk", bufs=4, space="PSUM") as psqk,
                tc.tile_pool(name="ps_v", bufs=2, space="PSUM") as psv,
            ):
                def load_w(nm, drt):
                    t = wp.tile([128, KI * 256], F16, tag=f"w{nm}",
                                name=f"w_{nm}")
                    nc.sync.dma_start(
                        t[:].rearrange("p (t o) -> p t o", t=KI),
                        drt[:, :].rearrange("(t p) o -> p t o", p=128),
                    )
                    return t

                def load_x(nb):
                    n0 = nb * NBLK
                    xc = []
                    for c in range(4):
                        xt = xp.tile([128, 4 * NBLK], F16, tag="x",
                                     name="xt")
                        nc.sync.dma_start(
                            xt[:].rearrange("p (t n) -> p t n", t=4),
                            xT[c * 512:(c + 1) * 512, n0:n0 + NBLK]
                            .rearrange("(t p) n -> p t n", p=128),
                        )
                        xc.append(xt)
                    return xc

                # DMA order: wq + first x block first so the PE starts early.
                w_sb = {"q": load_w("q", wqT)}
                xc0 = load_x(0)
                w_sb["k"] = load_w("k", wkT)
                w_sb["v"] = load_w("v", wvT)
                nc.sync.dma_start(cos_sb[:], cosT[:, :])
                nc.sync.dma_start(sin_sb[:], sinS[:, :])
                nc.sync.dma_start(ones_sb[:], ones_in[:, :])

                for nb in range(NB):
                    n0 = nb * NBLK
                    s0 = (nb % 4) * NBLK  # in-batch position offset
                    xc = xc0 if nb == 0 else load_x(nb)

                    for nm, outsb in (("q", q_sb), ("k", k_sb)):
                        for o in range(2):
                            pq = psqk.tile([128, NBLK], F32, tag="pqk")
                            for c in range(4):
                                for t in range(4):
                                    i = c * 4 + t
                                    nc.tensor.matmul(
                                        pq[:],
                                        w_sb[nm][:, i * 256 + o * 128:
                                                 i * 256 + o * 128 + 128]
                                        ,
                                        xc[c][:, t * NBLK:(t + 1) * NBLK]
                                        ,
                                        start=(i == 0), stop=(i == KI - 1),
                                    )
                            # RoPE: out = pq*cos + rot(pq)*sinS
                            t1 = rtp.tile([128, NBLK], F32, tag="t1")
                            nc.vector.tensor_mul(
                                t1[:], pq[:], cos_sb[:, s0:s0 + NBLK])
                            dst = outsb[o][:, n0:n0 + NBLK]
                            nc.vector.tensor_mul(
                                dst[0:64, :], pq[64:128, :],
                                sin_sb[0:64, s0:s0 + NBLK])
                            nc.vector.tensor_mul(
                                dst[64:128, :], pq[0:64, :],
                                sin_sb[64:128, s0:s0 + NBLK])
                            nc.vector.tensor_add(dst[:, :], dst[:, :], t1[:])

                    # v in natural layout: lhsT = x tile, rhs = wv
                    for j in range(4):
                        pv = psv.tile([128, 256], F32, tag="pv")
                        for c in range(4):
                            for t in range(4):
                                i = c * 4 + t
                                nc.tensor.matmul(
                                    pv[:],
                                    xc[c][:, t * NBLK + j * 128:
                                          t * NBLK + j * 128 + 128]
                                    ,
                                    w_sb["v"][:, i * 256:(i + 1) * 256]
                                    ,
                                    start=(i == 0), stop=(i == KI - 1),
                                )
                        jg = nb * 4 + j
                        nc.scalar.copy(v_sb[:, jg * 256:(jg + 1) * 256], pv[:])

            # ---------------- phase 2: attention ----------------
            # sk outer / sq inner: the softmax denominator accumulates on DVE
            # (fp16); its partition reduction is 4 single-partition ones.T@acc
            # matmuls on the PE into one PSUM tile (offsets 0/32/64/96), then
            # DVE reciprocal + GpSimd broadcast run off the PE critical path.
            # PSUM: 2x scores tiles [128,1024] (4 banks) + 4 po (4 banks).
            with (
                tc.tile_pool(name="ps_s", bufs=2, space="PSUM") as pss,
                tc.tile_pool(name="ps_o", bufs=1, space="PSUM") as pso,
                tc.tile_pool(name="e", bufs=4) as ep,
                tc.tile_pool(name="acc", bufs=2) as accp,
                tc.tile_pool(name="att_tmp", bufs=2) as atp,
            ):
                for b in range(2):
                    for h in range(2):
                        sq0 = b * 2048
                        po = [pso.tile([128, NBLK], F32, tag=f"po{q}",
                                       name=f"po{q}") for q in range(4)]
                        acc = [accp.tile([128, 2 * NBLK], F16, tag=f"acc{a}",
                                         name=f"acc{a}") for a in range(2)]
                        for sk in range(16):
                            kt = k_sb[h][:, b * 2048 + sk * 128:
                                         b * 2048 + sk * 128 + 128]
                            jg = b * 16 + sk
                            vt = v_sb[:, jg * 256 + h * 128:
                                      jg * 256 + h * 128 + 128]
                            es = []
                            for half in range(2):
                                ps = pss.tile([128, 2 * NBLK], F32, tag="ps")
                                for q in range(2):
                                    nc.tensor.matmul(
                                        ps[:, q * NBLK:(q + 1) * NBLK],
                                        kt,
                                        q_sb[h][:, sq0 + (half * 2 + q) * NBLK:
                                                sq0 + (half * 2 + q + 1) * NBLK],
                                        start=True, stop=True,
                                    )
                                e = ep.tile([128, 2 * NBLK], F16, tag="e")
                                nc.scalar.activation(e[:], ps[:], EXP,
                                                     scale=SCALE)
                                es.append(e)
                                # denominator partial sums on DVE (fp16 2x)
                                if sk == 0:
                                    nc.vector.tensor_copy(acc[half][:], e[:])
                                else:
                                    nc.vector.tensor_add(
                                        acc[half][:], acc[half][:], e[:])
                            for q in range(4):
                                nc.tensor.matmul(
                                    po[q][:],
                                    vt,
                                    es[q // 2][:, (q % 2) * NBLK:
                                               (q % 2 + 1) * NBLK],
                                    start=(sk == 0), stop=(sk == 15),
                                )
                        # denominator partition-reduce on PE: 4 matmuls
                        # den[32j, 0:512] = ones[128,1].T @ acc-slice
                        den = pss.tile([128, NBLK], F32, tag="ps",
                                       name="den")
                        for j in range(4):
                            nc.tensor.matmul(
                                den[32 * j:32 * j + 1, :],
                                ones_sb[:, 0:1],
                                acc[j // 2][:, (j % 2) * NBLK:
                                            (j % 2 + 1) * NBLK],
                                start=True, stop=True,
                                tile_position=(0, 32 * j),
                            )
                        # reciprocal straight out of PSUM (only partitions
                        # 0/32/64/96 carry data), then gather the 4 rows
                        # into a [1, 2048] row for the broadcast
                        rc = atp.tile([128, NBLK], F32, tag="rc", name="rc")
                        nc.vector.reciprocal(rc[:], den[:])
                        rrow = atp.tile([1, 4 * NBLK], F32,
                                        tag="rrow", name="rrow")
                        for j in range(4):
                            nc.sync.dma_start(
                                rrow[:, j * NBLK:(j + 1) * NBLK],
                                rc[32 * j:32 * j + 1, :],
                            )
                        for half in range(2):
                            bc = atp.tile([128, 2 * NBLK], F32,
                                          tag=f"bc{half}", name="bc")
                            nc.gpsimd.partition_broadcast(
                                bc[:], rrow[:, half * 1024:(half + 1) * 1024])
                            for q in range(2):
                                qq = half * 2 + q
                                osb = atp.tile([128, NBLK], F32,
                                               tag="osb", name="osb")
                                nc.vector.tensor_mul(
                                    osb[:], po[qq][:],
                                    bc[:, q * NBLK:(q + 1) * NBLK])
                                nc.sync.dma_start(
                                    outT[h * 128:(h + 1) * 128,
                                         sq0 + qq * NBLK:sq0 + (qq + 1) * NBLK],
                                    osb[:])
    nc.compile()
    return nc


def _get_nc():
    if "nc" not in _CACHE:
        _CACHE["nc"] = _build_nc()
    return _CACHE["nc"]


def _cos_sin():
    if "cs" not in _CACHE:
        half = np.arange(0, HEAD_DIM, 2, dtype=np.float32)[: HEAD_DIM // 2]
        freq = (1.0 / 10000.0 ** (half / HEAD_DIM)).astype(np.float32)
        t = np.arange(S, dtype=np.float32)
        freqs = np.outer(t, freq).astype(np.float32)  # [S, 64]
        emb = np.concatenate([freqs, freqs], axis=1)  # [S, 128]
        cosT = np.ascontiguousarray(np.cos(emb).astype(np.float32).T)
        sinT = np.ascontiguousarray(np.sin(emb).astype(np.float32).T)
        sinS = np.concatenate([-sinT[0:64], sinT[64:128]], axis=0)
        _CACHE["cs"] = (cosT, np.ascontiguousarray(sinS))
    return _CACHE["cs"]


def kernel(x, wq, wk, wv):
    global LAST_RESULT
    nc = _get_nc()
    cosT, sinS = _cos_sin()
    x2 = np.ascontiguousarray(
        np.asarray(x).reshape(NTOK, HIDDEN).T).astype(np.float16)
    ones8 = np.ones((128, 8), dtype=np.float16)
    in_maps = []
    for m in range(N_CORES):
        sl = slice(m * OPC, (m + 1) * OPC)
        in_maps.append({
            "xT": x2,
            "wqT": np.ascontiguousarray(np.asarray(wq)[sl].T).astype(np.float16),
            "wkT": np.ascontiguousarray(np.asarray(wk)[sl].T).astype(np.float16),
            "wvT": np.ascontiguousarray(np.asarray(wv)[sl].T).astype(np.float16),
            "cosT": cosT,
            "sinS": sinS,
            "ones_in": ones8,
        })
    res = run_bass_kernel_spmd(nc, in_maps, core_ids=list(range(N_CORES)))
    LAST_RESULT = res
    big = np.concatenate([r["outT"] for r in res.results], axis=0)
    return np.ascontiguousarray(big.T).reshape(B, S, HIDDEN).astype(np.float32)


if __name__ == "__main__":
    _get_nc()
    print("build OK")


# revision 7
# speedup vs baseline: 1.1343x; 1.0089x over previous
"""Llama RoPE attention (B=2, S=2048, H=2048, 16 heads) on 8 NeuronCores.

Tensor-parallel over heads: core m owns heads {2m, 2m+1}. Each core gets the
full activation x (transposed host-side to [HIDDEN, B*S], cast fp16) plus its
256-column slice of wq/wk/wv (host-transposed, fp16).

Schedule (single core): batch-0 projections first (P0), then batch-1
projections INTERLEAVED at fine grain with batch-0 attention groups so the
scalar engine's exp work hides under projection matmuls (INT), then the two
batch-1 attention groups back-to-back (C).

  projections: q_T/k_T = (w.T)^T @ x_T accumulated over 16 k-tiles (fp16),
  RoPE on DVE straight out of PSUM (fp16 out); v natural [n, d] fp16.
  attention per (batch, head): sk outer / sq inner; scores = k.T @ q, exp on
  scalar engine (fused 1/sqrt(d)) -> fp16 e; PV accumulates v.T @ e in PSUM;
  denominator partials on DVE (fp16); partition-reduce via 4 single-partition
  ones.T @ acc matmuls (PSUM partition offsets 0/32/64/96), reciprocal on
  DVE, partition_broadcast on GpSimd, scale + store.

PSUM: interleaved window = proj pool 2x2KB (qk+v shared) + scores 1x4KB +
po 4x2KB = 16KB; final window = scores 2x4KB + po 4x2KB = 16KB.
"""

import math
import os
import sys

for _p in ("/opt/trn_rl_repo", "/root/.axon_site/_ro/trn_rl_repo"):
    if os.path.isdir(_p) and _p not in sys.path:
        sys.path.insert(0, _p)
        break

import numpy as np

import concourse.bass as bass
import concourse.bacc as bacc
import concourse.mybir as mybir
from concourse import bass_isa, tile
from concourse.bass_utils import run_bass_kernel_spmd

N_CORES = 8
HIDDEN = 2048
N_HEAD = 16
HEAD_DIM = 128
B = 2
S = 2048
NTOK = B * S  # 4096
OPC = 256  # output cols per core = 2 heads * 128
KI = HIDDEN // 128  # 16 contraction tiles
NB = NTOK // 512  # 8 n-blocks of 512 tokens
NBLK = 512
SCALE = 1.0 / math.sqrt(HEAD_DIM)
F32 = mybir.dt.float32
F16 = mybir.dt.float16
EXP = mybir.ActivationFunctionType.Exp

_CACHE = {}

# test.py can read this after calling kernel() with BASS_TRACE=1
LAST_RESULT = None


def _build_nc():
    nc = bacc.Bacc("TRN2", target_bir_lowering=False, debug=False,
                   num_devices=N_CORES)
    xT = nc.dram_tensor("xT", [HIDDEN, NTOK], F16, kind="ExternalInput")
    wqT = nc.dram_tensor("wqT", [HIDDEN, OPC], F16, kind="ExternalInput")
    wkT = nc.dram_tensor("wkT", [HIDDEN, OPC], F16, kind="ExternalInput")
    wvT = nc.dram_tensor("wvT", [HIDDEN, OPC], F16, kind="ExternalInput")
    cosT = nc.dram_tensor("cosT", [HEAD_DIM, S], F32, kind="ExternalInput")
    sinS = nc.dram_tensor("sinS", [HEAD_DIM, S], F32, kind="ExternalInput")
    ones_in = nc.dram_tensor("ones_in", [128, 8], F16, kind="ExternalInput")
    outT = nc.dram_tensor("outT", [OPC, NTOK], F32, kind="ExternalOutput")

    with tile.TileContext(nc) as tc:
        with (
            tc.tile_pool(name="const", bufs=1) as cp,
            tc.tile_pool(name="qk_res", bufs=1) as qkp,
            tc.tile_pool(name="v_res", bufs=1) as vp,
            tc.tile_pool(name="x", bufs=6) as xp,
            tc.tile_pool(name="e", bufs=4) as ep,
            tc.tile_pool(name="acc", bufs=2) as accp,
            tc.tile_pool(name="att_tmp", bufs=2) as atp,
        ):
            # resident tensors
            q_sb = [qkp.tile([128, NTOK], F16, tag=f"q{o}", name=f"q_sb{o}")
                    for o in range(2)]
            k_sb = [qkp.tile([128, NTOK], F16, tag=f"k{o}", name=f"k_sb{o}")
                    for o in range(2)]
            v_sb = vp.tile([128, 32 * 256], F16, tag="v")  # [n%128, (nblk d)]
            cos_sb = cp.tile([128, S], F32, tag="cos")
            sin_sb = cp.tile([128, S], F32, tag="sin")
            ones_sb = cp.tile([128, 8], F16, tag="ones")

            w_sb = {}

            def load_w(nm, drt):
                # split into 4 chunks of 4 k-tiles so the first matmuls can
                # start before the whole weight is resident
                t = wp.tile([128, KI * 256], F16, tag=f"w{nm}", name=f"w_{nm}")
                for ch in range(4):
                    nc.sync.dma_start(
                        t[:, ch * 4 * 256:(ch + 1) * 4 * 256]
                        .rearrange("p (t o) -> p t o", t=4),
                        drt[ch * 512:(ch + 1) * 512, :]
                        .rearrange("(t p) o -> p t o", p=128),
                    )
                return t

            def load_x(nb):
                n0 = nb * NBLK
                xc = []
                for c in range(4):
                    xt = xp.tile([128, 4 * NBLK], F16, tag="x", name="xt")
                    nc.sync.dma_start(
                        xt[:].rearrange("p (t n) -> p t n", t=4),
                        xT[c * 512:(c + 1) * 512, n0:n0 + NBLK]
                        .rearrange("(t p) n -> p t n", p=128),
                    )
                    xc.append(xt)
                return xc

            # ---- projection unit emitters (one PSUM group each) ----
            def emit_qk_group(psq, nm, o, nb, xc):
                n0 = nb * NBLK
                s0 = (nb % 4) * NBLK
                pq = psq.tile([128, NBLK], F32, tag="pj")
                for c in range(4):
                    for t in range(4):
                        i = c * 4 + t
                        nc.tensor.matmul(
                            pq[:],
                            w_sb[nm][:, i * 256 + o * 128:
                                     i * 256 + o * 128 + 128],
                            xc[c][:, t * NBLK:(t + 1) * NBLK],
                            start=(i == 0), stop=(i == KI - 1),
                        )
                outsb = q_sb if nm == "q" else k_sb
                t1 = rtp.tile([128, NBLK], F32, tag="t1")
                nc.vector.tensor_mul(t1[:], pq[:], cos_sb[:, s0:s0 + NBLK])
                dst = outsb[o][:, n0:n0 + NBLK]
                nc.vector.tensor_mul(
                    dst[0:64, :], pq[64:128, :], sin_sb[0:64, s0:s0 + NBLK])
                nc.vector.tensor_mul(
                    dst[64:128, :], pq[0:64, :], sin_sb[64:128, s0:s0 + NBLK])
                nc.vector.tensor_add(dst[:, :], dst[:, :], t1[:])

            def emit_v_group(psq, j, nb, xc):
                pv = psq.tile([128, 256], F32, tag="pj")
                for c in range(4):
                    for t in range(4):
                        i = c * 4 + t
                        nc.tensor.matmul(
                            pv[:],
                            xc[c][:, t * NBLK + j * 128:
                                  t * NBLK + j * 128 + 128],
                            w_sb["v"][:, i * 256:(i + 1) * 256],
                            start=(i == 0), stop=(i == KI - 1),
                        )
                jg = nb * 4 + j
                nc.scalar.copy(v_sb[:, jg * 256:(jg + 1) * 256], pv[:])

            # ---- attention emitters ----
            def emit_sk_iter(pss, b, h, sk, po, acc, quarters=False):
                sq0 = b * 2048
                kt = k_sb[h][:, b * 2048 + sk * 128:b * 2048 + sk * 128 + 128]
                jg = b * 16 + sk
                vt = v_sb[:, jg * 256 + h * 128:jg * 256 + h * 128 + 128]
                if quarters:
                    # [128,512] score tiles / exps: 1 PSUM bank each, so the
                    # interleaved window fits proj(2) + scores(2) + po(4)
                    for q in range(4):
                        ps = pss.tile([128, NBLK], F32, tag="ps")
                        nc.tensor.matmul(
                            ps[:], kt,
                            q_sb[h][:, sq0 + q * NBLK:sq0 + (q + 1) * NBLK],
                            start=True, stop=True,
                        )
                        e = ep.tile([128, NBLK], F16, tag="e")
                        nc.scalar.activation(e[:], ps[:], EXP, scale=SCALE)
                        half, qh = q // 2, q % 2
                        dst = acc[half][:, qh * NBLK:(qh + 1) * NBLK]
                        if sk == 0:
                            nc.vector.tensor_copy(dst, e[:])
                        else:
                            nc.vector.tensor_add(dst, dst, e[:])
                        nc.tensor.matmul(
                            po[q][:], vt, e[:],
                            start=(sk == 0), stop=(sk == 15),
                        )
                    return
                es = []
                for half in range(2):
                    ps = pss.tile([128, 2 * NBLK], F32, tag="ps")
                    for q in range(2):
                        nc.tensor.matmul(
                            ps[:, q * NBLK:(q + 1) * NBLK],
                            kt,
                            q_sb[h][:, sq0 + (half * 2 + q) * NBLK:
                                    sq0 + (half * 2 + q + 1) * NBLK],
                            start=True, stop=True,
                        )
                    e = ep.tile([128, 2 * NBLK], F16, tag="e")
                    nc.scalar.activation(e[:], ps[:], EXP, scale=SCALE)
                    es.append(e)
                    if sk == 0:
                        nc.vector.tensor_copy(acc[half][:], e[:])
                    else:
                        nc.vector.tensor_add(acc[half][:], acc[half][:], e[:])
                for q in range(4):
                    nc.tensor.matmul(
                        po[q][:],
                        vt,
                        es[q // 2][:, (q % 2) * NBLK:(q % 2 + 1) * NBLK],
                        start=(sk == 0), stop=(sk == 15),
                    )

            def emit_epilogue(pss, b, h, po, acc):
                sq0 = b * 2048
                den = pss.tile([128, NBLK], F32, tag="ps", name="den")
                for j in range(4):
                    nc.tensor.matmul(
                        den[32 * j:32 * j + 1, :],
                        ones_sb[:, 0:1],
                        acc[j // 2][:, (j % 2) * NBLK:(j % 2 + 1) * NBLK],
                        start=True, stop=True,
                        tile_position=(0, 32 * j),
                    )
                rc = atp.tile([128, NBLK], F32, tag="rc", name="rc")
                nc.vector.reciprocal(rc[:], den[:])
                rrow = atp.tile([1, 4 * NBLK], F32, tag="rrow", name="rrow")
                for j in range(4):
                    nc.sync.dma_start(
                        rrow[:, j * NBLK:(j + 1) * NBLK],
                        rc[32 * j:32 * j + 1, :],
                    )
                for half in range(2):
                    bc = atp.tile([128, 2 * NBLK], F32, tag=f"bc{half}",
                                  name="bc")
                    nc.gpsimd.partition_broadcast(
                        bc[:], rrow[:, half * 1024:(half + 1) * 1024])
                    for q in range(2):
                        qq = half * 2 + q
                        osb = atp.tile([128, NBLK], F32, tag="osb",
                                       name="osb")
                        nc.vector.tensor_mul(
                            osb[:], po[qq][:], bc[:, q * NBLK:(q + 1) * NBLK])
                        nc.sync.dma_start(
                            outT[h * 128:(h + 1) * 128,
                                 sq0 + qq * NBLK:sq0 + (qq + 1) * NBLK],
                            osb[:])

            # =========== schedule ===========
            with (
                tc.tile_pool(name="w", bufs=1) as wp,
                tc.tile_pool(name="rope_tmp", bufs=3) as rtp,
            ):
                # priming DMAs: wq + first x block first
                w_sb["q"] = load_w("q", wqT)
                xcs = {0: load_x(0)}
                w_sb["k"] = load_w("k", wkT)
                w_sb["v"] = load_w("v", wvT)
                nc.sync.dma_start(cos_sb[:], cosT[:, :])
                nc.sync.dma_start(sin_sb[:], sinS[:, :])
                nc.sync.dma_start(ones_sb[:], ones_in[:, :])

                # ---- P0: batch-0 projections (nb 0..3) ----
                with tc.tile_pool(name="ps_p0", bufs=4, space="PSUM") as psq0:
                    for nb in range(4):
                        xc = xcs.pop(nb)
                        xcs[nb + 1] = load_x(nb + 1)
                        for o in range(2):
                            emit_qk_group(psq0, "q", o, nb, xc)
                        for o in range(2):
                            emit_qk_group(psq0, "k", o, nb, xc)
                        for j in range(4):
                            emit_v_group(psq0, j, nb, xc)

                # ---- INT: batch-1 projections woven with batch-0 attention
                with (
                    tc.tile_pool(name="ps_pj", bufs=2, space="PSUM") as psqI,
                    tc.tile_pool(name="ps_att", bufs=2, space="PSUM") as pssA,
                    tc.tile_pool(name="ps_po", bufs=1, space="PSUM") as poP,
                ):
                    po_g = {h: [poP.tile([128, NBLK], F32, tag=f"po{q}",
                                         name=f"po{q}") for q in range(4)]
                            for h in range(2)}
                    acc_g = {h: [accp.tile([128, 2 * NBLK], F16,
                                           tag=f"acc{a}", name=f"acc{a}")
                                 for a in range(2)]
                             for h in range(2)}
                    for s in range(4):
                        nb = 4 + s
                        xc = xcs.pop(nb)
                        if nb + 1 < NB:
                            xcs[nb + 1] = load_x(nb + 1)
                        h = 0 if s < 2 else 1
                        sk0 = 8 * (s % 2)
                        units = ([lambda j=j: emit_v_group(psqI, j, nb, xc)
                                  for j in range(2)]
                                 + [lambda o=o:
                                    emit_qk_group(psqI, "q", o, nb, xc)
                                    for o in range(2)]
                                 + [lambda o=o:
                                    emit_qk_group(psqI, "k", o, nb, xc)
                                    for o in range(2)]
                                 + [lambda j=j: emit_v_group(psqI, j, nb, xc)
                                    for j in range(2, 4)])
                        for u in range(8):
                            units[u]()
                            if s == 2 and u == 0:
                                # g(0,0) epilogue after its exps drain
                                emit_epilogue(pssA, 0, 0,
                                              po_g[0], acc_g[0])
                            emit_sk_iter(pssA, 0, h, sk0 + u,
                                         po_g[h], acc_g[h], quarters=True)
                    emit_epilogue(pssA, 0, 1, po_g[1], acc_g[1])

            # ---- C: batch-1 attention, double-buffered scores ----
            with (
                tc.tile_pool(name="ps_att2", bufs=2, space="PSUM") as pssC,
                tc.tile_pool(name="ps_po2", bufs=1, space="PSUM") as poC,
            ):
                for h in range(2):
                    po = [poC.tile([128, NBLK], F32, tag=f"po{q}",
                                   name=f"po{q}") for q in range(4)]
                    acc = [accp.tile([128, 2 * NBLK], F16, tag=f"acc{a}",
                                     name=f"acc{a}") for a in range(2)]
                    for sk in range(16):
                        emit_sk_iter(pssC, 1, h, sk, po, acc)
                    emit_epilogue(pssC, 1, h, po, acc)
    nc.compile()
    return nc


def _get_nc():
    if "nc" not in _CACHE:
        _CACHE["nc"] = _build_nc()
    return _CACHE["nc"]


def _cos_sin():
    if "cs" not in _CACHE:
        half = np.arange(0, HEAD_DIM, 2, dtype=np.float32)[: HEAD_DIM // 2]
        freq = (1.0 / 10000.0 ** (half / HEAD_DIM)).astype(np.float32)
        t = np.arange(S, dtype=np.float32)
        freqs = np.outer(t, freq).astype(np.float32)  # [S, 64]
        emb = np.concatenate([freqs, freqs], axis=1)  # [S, 128]
        cosT = np.ascontiguousarray(np.cos(emb).astype(np.float32).T)
        sinT = np.ascontiguousarray(np.sin(emb).astype(np.float32).T)
        sinS = np.concatenate([-sinT[0:64], sinT[64:128]], axis=0)
        _CACHE["cs"] = (cosT, np.ascontiguousarray(sinS))
    return _CACHE["cs"]


def kernel(x, wq, wk, wv):
    global LAST_RESULT
    nc = _get_nc()
    cosT, sinS = _cos_sin()
    x2 = np.ascontiguousarray(
        np.asarray(x).reshape(NTOK, HIDDEN).T).astype(np.float16)
    ones8 = np.ones((128, 8), dtype=np.float16)
    in_maps = []
    for m in range(N_CORES):
        sl = slice(m * OPC, (m + 1) * OPC)
        in_maps.append({
            "xT": x2,
            "wqT": np.ascontiguousarray(np.asarray(wq)[sl].T).astype(np.float16),
            "wkT": np.ascontiguousarray(np.asarray(wk)[sl].T).astype(np.float16),
            "wvT": np.ascontiguousarray(np.asarray(wv)[sl].T).astype(np.float16),
            "cosT": cosT,
            "sinS": sinS,
            "ones_in": ones8,
        })
    res = run_bass_kernel_spmd(nc, in_maps, core_ids=list(range(N_CORES)))
    LAST_RESULT = res
    big = np.concatenate([r["outT"] for r in res.results], axis=0)
    return np.ascontiguousarray(big.T).reshape(B, S, HIDDEN).astype(np.float32)


if __name__ == "__main__":
    _get_nc()
    print("build OK")


# revision 12
# speedup vs baseline: 1.1723x; 1.0335x over previous
"""Llama RoPE attention (B=2, S=2048, H=2048, 16 heads) on 8 NeuronCores.

Tensor-parallel over heads: core m owns heads {2m, 2m+1}. Each core gets the
full activation x (transposed host-side to [HIDDEN, B*S], cast fp16) plus its
256-column slice of wq/wk/wv (host-transposed, fp16).

Schedule (single core): batch-0 projections first (P0), then batch-1
projections INTERLEAVED at fine grain with batch-0 attention groups so the
scalar engine's exp work hides under projection matmuls (INT), then the two
batch-1 attention groups back-to-back (C).

  projections: q_T/k_T = (w.T)^T @ x_T accumulated over 16 k-tiles (fp16),
  RoPE on DVE straight out of PSUM (fp16 out); v natural [n, d] fp16.
  attention per (batch, head): sk outer / sq inner; scores = k.T @ q, exp on
  scalar engine (fused 1/sqrt(d)) -> fp16 e; PV accumulates v.T @ e in PSUM;
  denominator partials on DVE (fp16); partition-reduce via 4 single-partition
  ones.T @ acc matmuls (PSUM partition offsets 0/32/64/96), reciprocal on
  DVE, partition_broadcast on GpSimd, scale + store.

PSUM: interleaved window = proj pool 2x2KB (qk+v shared) + scores 1x4KB +
po 4x2KB = 16KB; final window = scores 2x4KB + po 4x2KB = 16KB.
"""

import math
import os
import sys

for _p in ("/opt/trn_rl_repo", "/root/.axon_site/_ro/trn_rl_repo"):
    if os.path.isdir(_p) and _p not in sys.path:
        sys.path.insert(0, _p)
        break

import numpy as np

import concourse.bass as bass
import concourse.bacc as bacc
import concourse.mybir as mybir
from concourse import bass_isa, tile
from concourse.bass_utils import run_bass_kernel_spmd

N_CORES = 8
HIDDEN = 2048
N_HEAD = 16
HEAD_DIM = 128
B = 2
S = 2048
NTOK = B * S  # 4096
OPC = 256  # output cols per core = 2 heads * 128
KI = HIDDEN // 128  # 16 contraction tiles
NB = NTOK // 512  # 8 n-blocks of 512 tokens
NBLK = 512
SCALE = 1.0 / math.sqrt(HEAD_DIM)
F32 = mybir.dt.float32
F16 = mybir.dt.float16
EXP = mybir.ActivationFunctionType.Exp

_CACHE = {}

# test.py can read this after calling kernel() with BASS_TRACE=1
LAST_RESULT = None


def _build_nc():
    nc = bacc.Bacc("TRN2", target_bir_lowering=False, debug=False,
                   num_devices=N_CORES)
    xT = nc.dram_tensor("xT", [HIDDEN, NTOK], F16, kind="ExternalInput")
    wqT = nc.dram_tensor("wqT", [HIDDEN, OPC], F16, kind="ExternalInput")
    wkT = nc.dram_tensor("wkT", [HIDDEN, OPC], F16, kind="ExternalInput")
    wvT = nc.dram_tensor("wvT", [HIDDEN, OPC], F16, kind="ExternalInput")
    cosT = nc.dram_tensor("cosT", [HEAD_DIM, S], F32, kind="ExternalInput")
    sinS = nc.dram_tensor("sinS", [HEAD_DIM, S], F32, kind="ExternalInput")
    ones_in = nc.dram_tensor("ones_in", [128, 8], F16, kind="ExternalInput")
    outT = nc.dram_tensor("outT", [OPC, NTOK], F32, kind="ExternalOutput")

    with tile.TileContext(nc) as tc:
        with (
            tc.tile_pool(name="const", bufs=1) as cp,
            tc.tile_pool(name="qk_res", bufs=1) as qkp,
            tc.tile_pool(name="v_res", bufs=1) as vp,
            tc.tile_pool(name="x", bufs=8) as xp,
            tc.tile_pool(name="e", bufs=4) as ep,
            tc.tile_pool(name="acc", bufs=2) as accp,
            tc.tile_pool(name="att_tmp", bufs=2) as atp,
        ):
            # resident tensors
            q_sb = [qkp.tile([128, NTOK], F16, tag=f"q{o}", name=f"q_sb{o}")
                    for o in range(2)]
            k_sb = [qkp.tile([128, NTOK], F16, tag=f"k{o}", name=f"k_sb{o}")
                    for o in range(2)]
            v_sb = vp.tile([128, 32 * 256], F16, tag="v")  # [n%128, (nblk d)]
            cos_sb = cp.tile([128, S], F32, tag="cos")
            sin_sb = cp.tile([128, S], F32, tag="sin")
            ones_sb = cp.tile([128, 8], F16, tag="ones")

            w_sb = {}

            def load_w(nm, drt):
                # split into 4 chunks of 4 k-tiles so the first matmuls can
                # start before the whole weight is resident
                t = wp.tile([128, KI * 256], F16, tag=f"w{nm}", name=f"w_{nm}")
                for ch in range(4):
                    nc.sync.dma_start(
                        t[:, ch * 4 * 256:(ch + 1) * 4 * 256]
                        .rearrange("p (t o) -> p t o", t=4),
                        drt[ch * 512:(ch + 1) * 512, :]
                        .rearrange("(t p) o -> p t o", p=128),
                    )
                return t

            def load_x(nb):
                n0 = nb * NBLK
                xc = []
                for c in range(4):
                    xt = xp.tile([128, 4 * NBLK], F16, tag="x", name="xt")
                    nc.sync.dma_start(
                        xt[:].rearrange("p (t n) -> p t n", t=4),
                        xT[c * 512:(c + 1) * 512, n0:n0 + NBLK]
                        .rearrange("(t p) n -> p t n", p=128),
                    )
                    xc.append(xt)
                return xc

            # ---- projection unit emitters (one PSUM group each) ----
            def emit_qk_group(psq, nm, o, nb, xc):
                n0 = nb * NBLK
                s0 = (nb % 4) * NBLK
                pq = psq.tile([128, NBLK], F32, tag="pj")
                for c in range(4):
                    for t in range(4):
                        i = c * 4 + t
                        nc.tensor.matmul(
                            pq[:],
                            w_sb[nm][:, i * 256 + o * 128:
                                     i * 256 + o * 128 + 128],
                            xc[c][:, t * NBLK:(t + 1) * NBLK],
                            start=(i == 0), stop=(i == KI - 1),
                        )
                outsb = q_sb if nm == "q" else k_sb
                t1 = rtp.tile([128, NBLK], F32, tag="t1")
                nc.vector.tensor_mul(t1[:], pq[:], cos_sb[:, s0:s0 + NBLK])
                dst = outsb[o][:, n0:n0 + NBLK]
                nc.vector.tensor_mul(
                    dst[0:64, :], pq[64:128, :], sin_sb[0:64, s0:s0 + NBLK])
                nc.vector.tensor_mul(
                    dst[64:128, :], pq[0:64, :], sin_sb[64:128, s0:s0 + NBLK])
                nc.vector.tensor_add(dst[:, :], dst[:, :], t1[:])

            def emit_v_group(psq, j, nb, xc):
                pv = psq.tile([128, 256], F32, tag="pj")
                for c in range(4):
                    for t in range(4):
                        i = c * 4 + t
                        nc.tensor.matmul(
                            pv[:],
                            xc[c][:, t * NBLK + j * 128:
                                  t * NBLK + j * 128 + 128],
                            w_sb["v"][:, i * 256:(i + 1) * 256],
                            start=(i == 0), stop=(i == KI - 1),
                        )
                jg = nb * 4 + j
                nc.scalar.copy(v_sb[:, jg * 256:(jg + 1) * 256], pv[:])

            # ---- attention emitters ----
            def emit_sk_iter(pss, b, h, sk, po, acc, quarters=False):
                sq0 = b * 2048
                kt = k_sb[h][:, b * 2048 + sk * 128:b * 2048 + sk * 128 + 128]
                jg = b * 16 + sk
                vt = v_sb[:, jg * 256 + h * 128:jg * 256 + h * 128 + 128]
                if quarters:
                    # [128,512] score tiles / exps: 1 PSUM bank each, so the
                    # interleaved window fits proj(2) + scores(2) + po(4)
                    for q in range(4):
                        ps = pss.tile([128, NBLK], F32, tag="ps")
                        nc.tensor.matmul(
                            ps[:], kt,
                            q_sb[h][:, sq0 + q * NBLK:sq0 + (q + 1) * NBLK],
                            start=True, stop=True,
                        )
                        e = ep.tile([128, NBLK], F16, tag="e")
                        nc.scalar.activation(e[:], ps[:], EXP, scale=SCALE)
                        half, qh = q // 2, q % 2
                        dst = acc[half][:, qh * NBLK:(qh + 1) * NBLK]
                        if sk == 0:
                            nc.vector.tensor_copy(dst, e[:])
                        else:
                            nc.vector.tensor_add(dst, dst, e[:])
                        nc.tensor.matmul(
                            po[q][:], vt, e[:],
                            start=(sk == 0), stop=(sk == 15),
                        )
                    return
                es = []
                for half in range(2):
                    ps = pss.tile([128, 2 * NBLK], F32, tag="ps")
                    for q in range(2):
                        nc.tensor.matmul(
                            ps[:, q * NBLK:(q + 1) * NBLK],
                            kt,
                            q_sb[h][:, sq0 + (half * 2 + q) * NBLK:
                                    sq0 + (half * 2 + q + 1) * NBLK],
                            start=True, stop=True,
                        )
                    e = ep.tile([128, 2 * NBLK], F16, tag="e")
                    nc.scalar.activation(e[:], ps[:], EXP, scale=SCALE)
                    es.append(e)
                    if sk == 0:
                        nc.vector.tensor_copy(acc[half][:], e[:])
                    else:
                        nc.vector.tensor_add(acc[half][:], acc[half][:], e[:])
                for q in range(4):
                    nc.tensor.matmul(
                        po[q][:],
                        vt,
                        es[q // 2][:, (q % 2) * NBLK:(q % 2 + 1) * NBLK],
                        start=(sk == 0), stop=(sk == 15),
                    )

            def emit_epilogue(pss, b, h, po, acc):
                sq0 = b * 2048
                den = pss.tile([128, NBLK], F32, tag="ps", name="den")
                for j in range(4):
                    nc.tensor.matmul(
                        den[32 * j:32 * j + 1, :],
                        ones_sb[:, 0:1],
                        acc[j // 2][:, (j % 2) * NBLK:(j % 2 + 1) * NBLK],
                        start=True, stop=True,
                        tile_position=(0, 32 * j),
                    )
                rc = atp.tile([128, NBLK], F32, tag="rc", name="rc")
                nc.vector.reciprocal(rc[:], den[:])
                rrow = atp.tile([1, 4 * NBLK], F32, tag="rrow", name="rrow")
                for j in range(4):
                    nc.sync.dma_start(
                        rrow[:, j * NBLK:(j + 1) * NBLK],
                        rc[32 * j:32 * j + 1, :],
                    )
                for half in range(2):
                    bc = atp.tile([128, 2 * NBLK], F32, tag=f"bc{half}",
                                  name="bc")
                    nc.gpsimd.partition_broadcast(
                        bc[:], rrow[:, half * 1024:(half + 1) * 1024])
                    for q in range(2):
                        qq = half * 2 + q
                        osb = atp.tile([128, NBLK], F32, tag="osb",
                                       name="osb")
                        nc.vector.tensor_mul(
                            osb[:], po[qq][:], bc[:, q * NBLK:(q + 1) * NBLK])
                        nc.sync.dma_start(
                            outT[h * 128:(h + 1) * 128,
                                 sq0 + qq * NBLK:sq0 + (qq + 1) * NBLK],
                            osb[:])

            # =========== schedule ===========
            with (
                tc.tile_pool(name="w", bufs=1) as wp,
                tc.tile_pool(name="rope_tmp", bufs=3) as rtp,
            ):
                # priming DMAs: interleave wq chunks with the first x block's
                # chunks so the very first matmuls are gated by ~0.75MB
                wq_t = wp.tile([128, KI * 256], F16, tag="wq", name="w_q")
                xc0 = [xp.tile([128, 4 * NBLK], F16, tag="x", name="xt")
                       for _ in range(4)]
                for ch in range(4):
                    nc.sync.dma_start(
                        wq_t[:, ch * 4 * 256:(ch + 1) * 4 * 256]
                        .rearrange("p (t o) -> p t o", t=4),
                        wqT[ch * 512:(ch + 1) * 512, :]
                        .rearrange("(t p) o -> p t o", p=128),
                    )
                    nc.sync.dma_start(
                        xc0[ch][:].rearrange("p (t n) -> p t n", t=4),
                        xT[ch * 512:(ch + 1) * 512, 0:NBLK]
                        .rearrange("(t p) n -> p t n", p=128),
                    )
                w_sb["q"] = wq_t
                xcs = {0: xc0}
                w_sb["k"] = load_w("k", wkT)
                w_sb["v"] = load_w("v", wvT)
                nc.sync.dma_start(cos_sb[:], cosT[:, :])
                nc.sync.dma_start(sin_sb[:], sinS[:, :])
                nc.sync.dma_start(ones_sb[:], ones_in[:, :])

                # ---- P0: batch-0 projections (nb 0..3) ----
                with tc.tile_pool(name="ps_p0", bufs=4, space="PSUM") as psq0:
                    for nb in range(4):
                        xc = xcs.pop(nb)
                        xcs[nb + 1] = load_x(nb + 1)
                        for o in range(2):
                            emit_qk_group(psq0, "q", o, nb, xc)
                        for o in range(2):
                            emit_qk_group(psq0, "k", o, nb, xc)
                        for j in range(4):
                            emit_v_group(psq0, j, nb, xc)

                # ---- INT: batch-1 projections woven with batch-0 attention
                with (
                    tc.tile_pool(name="ps_pj", bufs=2, space="PSUM") as psqI,
                    tc.tile_pool(name="ps_att", bufs=2, space="PSUM") as pssA,
                    tc.tile_pool(name="ps_po", bufs=1, space="PSUM") as poP,
                ):
                    po_g = {h: [poP.tile([128, NBLK], F32, tag=f"po{q}",
                                         name=f"po{q}") for q in range(4)]
                            for h in range(2)}
                    acc_g = {h: [accp.tile([128, 2 * NBLK], F16,
                                           tag=f"acc{a}", name=f"acc{a}")
                                 for a in range(2)]
                             for h in range(2)}
                    for s in range(4):
                        nb = 4 + s
                        xc = xcs.pop(nb)
                        if nb + 1 < NB:
                            xcs[nb + 1] = load_x(nb + 1)
                        h = 0 if s < 2 else 1
                        sk0 = 8 * (s % 2)
                        units = ([lambda j=j: emit_v_group(psqI, j, nb, xc)
                                  for j in range(2)]
                                 + [lambda o=o:
                                    emit_qk_group(psqI, "q", o, nb, xc)
                                    for o in range(2)]
                                 + [lambda o=o:
                                    emit_qk_group(psqI, "k", o, nb, xc)
                                    for o in range(2)]
                                 + [lambda j=j: emit_v_group(psqI, j, nb, xc)
                                    for j in range(2, 4)])
                        for u in range(8):
                            units[u]()
                            if s == 2:
                                if u == 0:
                                    # g(0,0) epilogue; g(0,1)'s first PV is
                                    # delayed a unit so the chain can free po
                                    emit_epilogue(pssA, 0, 0,
                                                  po_g[0], acc_g[0])
                                else:
                                    emit_sk_iter(pssA, 0, h, sk0 + u - 1,
                                                 po_g[h], acc_g[h],
                                                 quarters=True)
                            else:
                                emit_sk_iter(pssA, 0, h, sk0 + u,
                                             po_g[h], acc_g[h],
                                             quarters=True)
                        if s == 2:
                            emit_sk_iter(pssA, 0, h, sk0 + 7,
                                         po_g[h], acc_g[h], quarters=True)
                    emit_epilogue(pssA, 0, 1, po_g[1], acc_g[1])

            # ---- C: batch-1 attention, double-buffered scores ----
            with (
                tc.tile_pool(name="ps_att2", bufs=2, space="PSUM") as pssC,
                tc.tile_pool(name="ps_po2", bufs=1, space="PSUM") as poC,
            ):
                for h in range(2):
                    po = [poC.tile([128, NBLK], F32, tag=f"po{q}",
                                   name=f"po{q}") for q in range(4)]
                    acc = [accp.tile([128, 2 * NBLK], F16, tag=f"acc{a}",
                                     name=f"acc{a}") for a in range(2)]
                    for sk in range(16):
                        emit_sk_iter(pssC, 1, h, sk, po, acc)
                    emit_epilogue(pssC, 1, h, po, acc)
    nc.compile()
    return nc


def _get_nc():
    if "nc" not in _CACHE:
        _CACHE["nc"] = _build_nc()
    return _CACHE["nc"]


def _cos_sin():
    if "cs" not in _CACHE:
        half = np.arange(0, HEAD_DIM, 2, dtype=np.float32)[: HEAD_DIM // 2]
        freq = (1.0 / 10000.0 ** (half / HEAD_DIM)).astype(np.float32)
        t = np.arange(S, dtype=np.float32)
        freqs = np.outer(t, freq).astype(np.float32)  # [S, 64]
        emb = np.concatenate([freqs, freqs], axis=1)  # [S, 128]
        cosT = np.ascontiguousarray(np.cos(emb).astype(np.float32).T)
        sinT = np.ascontiguousarray(np.sin(emb).astype(np.float32).T)
        sinS = np.concatenate([-sinT[0:64], sinT[64:128]], axis=0)
        _CACHE["cs"] = (cosT, np.ascontiguousarray(sinS))
    return _CACHE["cs"]


def kernel(x, wq, wk, wv):
    global LAST_RESULT
    nc = _get_nc()
    cosT, sinS = _cos_sin()
    x2 = np.ascontiguousarray(
        np.asarray(x).reshape(NTOK, HIDDEN).T).astype(np.float16)
    ones8 = np.ones((128, 8), dtype=np.float16)
    in_maps = []
    for m in range(N_CORES):
        sl = slice(m * OPC, (m + 1) * OPC)
        in_maps.append({
            "xT": x2,
            "wqT": np.ascontiguousarray(np.asarray(wq)[sl].T).astype(np.float16),
            "wkT": np.ascontiguousarray(np.asarray(wk)[sl].T).astype(np.float16),
            "wvT": np.ascontiguousarray(np.asarray(wv)[sl].T).astype(np.float16),
            "cosT": cosT,
            "sinS": sinS,
            "ones_in": ones8,
        })
    res = run_bass_kernel_spmd(nc, in_maps, core_ids=list(range(N_CORES)))
    LAST_RESULT = res
    big = np.concatenate([r["outT"] for r in res.results], axis=0)
    return np.ascontiguousarray(big.T).reshape(B, S, HIDDEN).astype(np.float32)


if __name__ == "__main__":
    _get_nc()
    print("build OK")
